# revision 1
# baseline (speedup 1.0000x reference)
"""Trainium2 Bass kernel for nn_BaseDTA (quadrant dual-token attention).

Data-parallel over batch: each of the 8 NeuronCores processes one sample
end-to-end (4 quadrant MHSA sequences of length 1026 + gating + a second
4096x256 cross-attention). No collectives.

Layout strategy: channels-on-partitions ([C, T]) throughout.
 - Q/K projections produce qT/kT [C, T] directly (lhsT = W^T tiles).
 - V projection produces token-major V [T, C] (lhsT = Y token tiles).
 - Scores are computed transposed, S^T [keys, queries], with 4 heads
   row-packed into the PE array (tile_position=(32j, 0)).
 - exp runs on ACT straight out of PSUM into SBUF (float32r).
 - AV is col-packed (tile_position=(0, 32j)) so the 4 heads' outputs land
   stacked on partitions = channel layout for the O-projection.
 - softmax denominators via ones-matmul with M=32/128 replication so the
   reciprocal broadcast needs no partition games.
All matmuls run in float32r (1 cycle/row at N>=256 vs 4 for fp32).
"""

import math

import numpy as np

import concourse.bass as bass
import concourse.mybir as mybir
import concourse.tile as tile
from concourse import bacc
from concourse.bass_utils import run_bass_kernel_spmd

F32 = mybir.dt.float32
F32R = mybir.dt.float32r
BF16 = mybir.dt.bfloat16
AF = mybir.ActivationFunctionType
AX = mybir.AxisListType
ALU = mybir.AluOpType

B, C, H, W = 8, 256, 64, 64
h2, w2 = H // 2, W // 2          # 32
NH = 8
HD = C // NH                     # head dim 32
HW = H * W                       # 4096
PIX = h2 * w2                    # 1024
T = PIX + 2                      # 1026 tokens (1024 pixels + lt + gt)
QC = 342                         # query chunk: 1026 = 3*342
NKT = 9                          # key tiles: 8*128 + 2
QC2 = 512                        # attn2 query chunk (4096 = 8*512)
CHUNKS = [(0, 256), (256, 256), (512, 256), (768, 256), (1024, 2)]


def _build():
    nc = bacc.Bacc(trn_type="TRN2", target_bir_lowering=False, num_devices=8)

    x_d = nc.dram_tensor("x", [C, HW], F32, kind="ExternalInput")
    w_names = ["wq_t", "wk_t", "wv_t", "wo_t", "wfuse_t"]
    w_d = {n: nc.dram_tensor(n, [C, C], F32, kind="ExternalInput") for n in w_names}
    b_names = ["bq", "bk", "bo", "bfuse"]
    b_d = {n: nc.dram_tensor(n, [C, 1], F32, kind="ExternalInput") for n in b_names}
    bv_rep_d = nc.dram_tensor("bv_rep", [128, C], F32, kind="ExternalInput")
    bfuse_rep_d = nc.dram_tensor("bfuse_rep", [128, C], F32, kind="ExternalInput")
    g_names = ["wrow_rep", "brow_rep", "wcol_rep", "bcol_rep"]
    g_d = {n: nc.dram_tensor(n, [128, h2], F32, kind="ExternalInput") for n in g_names}
    wgt_rep_d = nc.dram_tensor("wgt_rep", [128, H], F32, kind="ExternalInput")
    bgt_rep_d = nc.dram_tensor("bgt_rep", [128, H], F32, kind="ExternalInput")
    o_d = nc.dram_tensor("o", [C, HW], F32, kind="ExternalOutput")

    with tile.TileContext(nc) as tc:
        _emit(nc, tc, x_d, w_d, b_d, bv_rep_d, bfuse_rep_d, g_d, wgt_rep_d,
              bgt_rep_d, o_d)
    nc.compile()
    return nc


def _emit(nc, tc, x_d, w_d, b_d, bv_rep_d, bfuse_rep_d, g_d, wgt_rep_d,
          bgt_rep_d, o_d):
    with tc.tile_pool(name="singles", bufs=1) as singles:
        # ---- tiles that live for the whole kernel ------------------------
        FW = singles.tile([128, 2, HW], F32R)        # f_wlt, filled per quad
        RP = singles.tile([128, 2, 4 * H], F32R)     # conv_fuse rhs (cols hh*4+q)
        WF = singles.tile([128, 2, C], F32R)
        ONES = singles.tile([128, 128], F32R)
        BF = singles.tile([128, 2, 1], F32)
        BFr = singles.tile([128, C], F32)

        with tc.tile_pool(name="stage", bufs=2) as stage:
            st = stage.tile([128, 2, C], F32, tag="wstage")
            for ct in range(2):
                nc.sync.dma_start(out=st[:, ct, :],
                                  in_=w_d["wfuse_t"][ct * 128:(ct + 1) * 128, :])
            nc.vector.tensor_copy(WF[:, :, :], st[:, :, :])
            ost = stage.tile([128, 128], F32, tag="ones_stage")
            nc.vector.memset(ost[:, :], 1.0)
            nc.vector.tensor_copy(ONES[:, :], ost[:, :])
        for ct in range(2):
            nc.sync.dma_start(out=BF[:, ct, :],
                              in_=b_d["bfuse"][ct * 128:(ct + 1) * 128, :])
        nc.sync.dma_start(out=BFr[:, :], in_=bfuse_rep_d[:, :])

        _emit_quads(nc, tc, x_d, w_d, b_d, bv_rep_d, g_d, wgt_rep_d, bgt_rep_d,
                    FW, RP, ONES)
        _emit_attn2(nc, tc, FW, RP, WF, ONES, BF, BFr, o_d)


def _emit_quads(nc, tc, x_d, w_d, b_d, bv_rep_d, g_d, wgt_rep_d, bgt_rep_d,
                FW, RP, ONES):
    with (
        tc.tile_pool(name="p1", bufs=1) as p1,
        tc.tile_pool(name="stage1", bufs=2) as stage,
        tc.tile_pool(name="qpool", bufs=1) as qpool,
        tc.tile_pool(name="ykpool", bufs=2) as ykpool,
        tc.tile_pool(name="apool", bufs=4) as apool,
        tc.tile_pool(name="gpool", bufs=1) as gpool,
        tc.tile_pool(name="ps_small", bufs=1, space="PSUM") as ps_small,
        tc.tile_pool(name="ps_s", bufs=1, space="PSUM") as ps_s,
    ):
        X = p1.tile([128, 2, HW], F32)
        WQ = p1.tile([128, 2, C], F32R)
        WK = p1.tile([128, 2, C], F32R)
        WV = p1.tile([128, 2, C], F32R)
        WO = p1.tile([128, 2, C], F32R)
        BQ = p1.tile([128, 2, 1], F32)
        BK = p1.tile([128, 2, 1], F32)
        BO = p1.tile([128, 2, 1], F32)
        BVr = p1.tile([128, C], F32)
        WRr = p1.tile([128, h2], F32)
        BRr = p1.tile([128, h2], F32)
        WCr = p1.tile([128, h2], F32)
        BCr = p1.tile([128, h2], F32)
        WGr = p1.tile([128, H], F32)
        BGr = p1.tile([128, H], F32)
        GT = p1.tile([128, 2, 1], F32)
        ONESB = p1.tile([128, 32], BF16)
        nc.vector.memset(ONESB[:, :], 1.0)

        for ct in range(2):
            for xc in range(4):
                nc.sync.dma_start(
                    out=X[:, ct, xc * 1024:(xc + 1) * 1024],
                    in_=x_d[ct * 128:(ct + 1) * 128, xc * 1024:(xc + 1) * 1024])
        for name, dst in [("wq_t", WQ), ("wk_t", WK), ("wv_t", WV),
                          ("wo_t", WO)]:
            st = stage.tile([128, 2, C], F32, tag="wstage1")
            for ct in range(2):
                nc.sync.dma_start(out=st[:, ct, :],
                                  in_=w_d[name][ct * 128:(ct + 1) * 128, :])
            nc.vector.tensor_copy(dst[:, :, :], st[:, :, :])
        for name, dst in [("bq", BQ), ("bk", BK), ("bo", BO)]:
            for ct in range(2):
                nc.sync.dma_start(out=dst[:, ct, :],
                                  in_=b_d[name][ct * 128:(ct + 1) * 128, :])
        nc.sync.dma_start(out=BVr[:, :], in_=bv_rep_d[:, :])
        for name, dst in [("wrow_rep", WRr), ("brow_rep", BRr),
                          ("wcol_rep", WCr), ("bcol_rep", BCr)]:
            nc.sync.dma_start(out=dst[:, :], in_=g_d[name][:, :])
        nc.sync.dma_start(out=WGr[:, :], in_=wgt_rep_d[:, :])
        nc.sync.dma_start(out=BGr[:, :], in_=bgt_rep_d[:, :])

        GTP = p1.tile([128, 2, 4], F32)
        for ct in range(2):
            for xc in range(4):
                scr = gpool.tile([128, 1024], F32, tag="scr")
                nc.scalar.activation(out=scr[:, :],
                                     in_=X[:, ct, xc * 1024:(xc + 1) * 1024],
                                     func=AF.Copy,
                                     accum_out=GTP[:, ct, xc:xc + 1])
            nc.vector.reduce_sum(GT[:, ct, :], GTP[:, ct, :], AX.X)
            nc.vector.tensor_scalar_mul(GT[:, ct, :], GT[:, ct, :], 1.0 / HW)

        quad_tiles = {}

        def prep(q):
            """Yield-per-chunk emission of Y build + Q/K/V projections."""
            r0, c0 = h2 * (q // 2), w2 * (q % 2)
            Y = ykpool.tile([128, 2, T], F32R, tag="Y")
            LT = gpool.tile([128, 2, 1], F32, tag="LT")
            for ct in range(2):
                xv = X[:, ct, :].rearrange("p (a b) -> p a b", a=H)[
                    :, r0:r0 + h2, c0:c0 + w2]
                yq = Y[:, ct, 0:PIX].rearrange("p (a b) -> p a b", a=h2)
                nc.vector.tensor_copy(yq, xv)
                nc.vector.reduce_sum(LT[:, ct, :], Y[:, ct, 0:PIX].bitcast(F32),
                                     AX.X)
                nc.vector.tensor_scalar_mul(LT[:, ct, :], LT[:, ct, :],
                                            1.0 / PIX)
                nc.vector.tensor_copy(Y[:, ct, PIX:PIX + 1], LT[:, ct, :])
                nc.vector.tensor_copy(Y[:, ct, PIX + 1:T], GT[:, ct, :])
                yield
            QT = ykpool.tile([128, 2, T], F32R, tag="QT")
            KT = ykpool.tile([128, 2, T], F32R, tag="KT")
            quad_tiles[q] = [Y, QT, KT, None]
            for Wt, Bt, dst in [(WQ, BQ, QT), (WK, BK, KT)]:
                for mt in range(2):
                    for qs in range(0, T, QC):
                        pq = ps_small.tile([128, QC], F32, tag="pq")
                        for kt2 in range(2):
                            nc.tensor.matmul(
                                pq[:, :], Wt[:, kt2, mt * 128:(mt + 1) * 128],
                                Y[:, kt2, qs:qs + QC],
                                start=(kt2 == 0), stop=(kt2 == 1))
                        nc.vector.tensor_scalar_add(dst[:, mt, qs:qs + QC],
                                                    pq[:, :], Bt[:, mt, :])
                        yield
            V8 = ykpool.tile([128, NKT, C], BF16, tag="V8")
            quad_tiles[q][3] = V8
            for tt in range(NKT):
                n = 128 if tt < 8 else T - 8 * 128
                pv = ps_small.tile([128, C], F32, tag="pv")
                for kt2 in range(2):
                    nc.tensor.matmul(pv[0:n, :],
                                     Y[:, kt2, tt * 128:tt * 128 + n],
                                     WV[:, kt2, :],
                                     start=(kt2 == 0), stop=(kt2 == 1))
                nc.vector.tensor_add(V8[0:n, tt, :], pv[0:n, :], BVr[0:n, :])
                yield

        def att(q):
            """Yield-per-chunk emission of attention + O-proj + gates."""
            r0, c0 = h2 * (q // 2), w2 * (q % 2)
            Y, QT, KT, V8 = quad_tiles[q]
            AT = qpool.tile([128, 2, T], F32R, tag="AT")
            OT = qpool.tile([128, 2, T], F32, tag="OT")
            for gr in range(2):
                for qs in range(0, T, QC):
                    qn = QC
                    avp = ps_small.tile([128, QC], F32, tag="avp")
                    dp = ps_small.tile([128, QC2], F32, tag="dp")

                    def scores_exp(kt):
                        n = 128 if kt < 8 else T - 8 * 128
                        ks = kt * 128
                        sp = ps_s.tile([128, 4, 512], F32, tag="sp")
                        for j in range(4):
                            nc.tensor.matmul(
                                sp[0:n, j, 0:qn],
                                KT[32 * j:32 * j + 32, gr, ks:ks + n],
                                QT[32 * j:32 * j + 32, gr, qs:qs + qn],
                                start=True, stop=True,
                                tile_position=(32 * j, 0))
                        At = apool.tile([128, 4, QC], BF16, tag="At")
                        nc.scalar.activation(out=At[0:n, :, 0:qn],
                                             in_=sp[0:n, :, 0:qn], func=AF.Exp)
                        return At

                    def av_d(kt, At):
                        n = 128 if kt < 8 else T - 8 * 128
                        for j in range(4):
                            hh = 4 * gr + j
                            nc.tensor.matmul(
                                avp[32 * j:32 * j + 32, 0:qn],
                                V8[0:n, kt, 32 * hh:32 * hh + 32],
                                At[0:n, j, 0:qn],
                                start=(kt == 0), stop=(kt == 8),
                                tile_position=(0, 32 * j))
                            nc.tensor.matmul(
                                dp[32 * j:32 * j + 32, 0:qn],
                                ONESB[0:n, :],
                                At[0:n, j, 0:qn],
                                start=(kt == 0), stop=(kt == 8),
                                tile_position=(0, 32 * j))

                    prev = None
                    for kt in range(NKT):
                        At = scores_exp(kt)
                        if prev is not None:
                            av_d(prev[0], prev[1])
                        prev = (kt, At)
                    av_d(prev[0], prev[1])
                    dr = gpool.tile([128, QC], F32, tag="dr")
                    nc.vector.reciprocal(dr[:, 0:qn], dp[:, 0:qn])
                    nc.vector.tensor_mul(AT[:, gr, qs:qs + qn], avp[:, 0:qn],
                                         dr[:, 0:qn])
                    if gr == 1:
                        # both head-group halves of this query chunk are done:
                        # O-projection + residual can overlap remaining blocks
                        for mt in range(2):
                            po = ps_small.tile([128, QC], F32, tag="pq")
                            for kt2 in range(2):
                                nc.tensor.matmul(
                                    po[:, :],
                                    WO[:, kt2, mt * 128:(mt + 1) * 128],
                                    AT[:, kt2, qs:qs + QC],
                                    start=(kt2 == 0), stop=(kt2 == 1))
                            nc.vector.scalar_tensor_tensor(
                                OT[:, mt, qs:qs + QC], po[:, :], BO[:, mt, :],
                                Y[:, mt, qs:qs + QC].bitcast(F32),
                                op0=ALU.add, op1=ALU.add)
                    yield

            for ct in range(2):
                ltp = OT[:, ct, PIX:PIX + 1]
                gtp = OT[:, ct, PIX + 1:T]
                row = gpool.tile([128, h2], F32, tag="row")
                col = gpool.tile([128, h2], F32, tag="col")
                nc.vector.scalar_tensor_tensor(row[:, :], WRr[:, :], ltp,
                                               BRr[:, :], op0=ALU.mult,
                                               op1=ALU.add)
                nc.vector.scalar_tensor_tensor(col[:, :], WCr[:, :], ltp,
                                               BCr[:, :], op0=ALU.mult,
                                               op1=ALU.add)
                prod = gpool.tile([128, h2, w2], F32, tag="prod")
                nc.vector.tensor_mul(
                    prod[:, :, :],
                    row[:, :, None].broadcast_to([128, h2, w2]),
                    col[:, None, :].broadcast_to([128, h2, w2]))
                eg = gpool.tile([128, h2, w2], F32, tag="eg")
                nc.scalar.activation(out=eg[:, :, :], in_=prod[:, :, :],
                                     func=AF.Exp)
                # sigmoid(z) = e/(1+e): stays on the Exp ACT table
                e1 = gpool.tile([128, h2, w2], F32, tag="e1")
                nc.vector.tensor_scalar_add(e1[:, :, :], eg[:, :, :], 1.0)
                nc.vector.reciprocal(e1[:, :, :], e1[:, :, :])
                nc.vector.tensor_mul(eg[:, :, :], eg[:, :, :], e1[:, :, :])
                fv = FW[:, ct, :].rearrange("p (a b) -> p a b", a=H)[
                    :, r0:r0 + h2, c0:c0 + w2]
                xp = OT[:, ct, 0:PIX].rearrange("p (a b) -> p a b", a=h2)
                nc.vector.tensor_mul(fv, xp, eg[:, :, :])
                rp_v = RP[:, ct, :].rearrange("p (a b) -> p a b", b=4)[:, :, q]
                nc.vector.scalar_tensor_tensor(rp_v, WGr[:, :], gtp, BGr[:, :],
                                               op0=ALU.mult, op1=ALU.add)
                yield

        def drain(g):
            if g is None:
                return
            for _ in g:
                pass

        drain(prep(0))
        for q in range(4):
            a = att(q)
            p = prep(q + 1) if q < 3 else None
            while True:
                try:
                    next(a)
                except StopIteration:
                    break
                if p is not None:
                    for _ in range(2):
                        try:
                            next(p)
                        except StopIteration:
                            p = None
                            break
            drain(p)


def _emit_attn2(nc, tc, FW, RP, WF, ONES, BF, BFr, o_d):
    with (
        tc.tile_pool(name="a2pool", bufs=1) as a2pool,
        tc.tile_pool(name="opool", bufs=2) as opool,
        tc.tile_pool(name="ps2_small", bufs=1, space="PSUM") as ps2_small,
        tc.tile_pool(name="ps2_k", bufs=2, space="PSUM") as ps2_k,
        tc.tile_pool(name="ps_s2", bufs=1, space="PSUM") as ps_s2,
    ):
        K2T = a2pool.tile([128, 2, C], F32R, tag="K2T")   # [c, keys]
        K2K = a2pool.tile([128, 2, C], F32R, tag="K2K")   # [keys, c]
        for mt in range(2):
            pk = ps2_k.tile([128, C], F32, tag="pk")
            for kt2 in range(2):
                nc.tensor.matmul(pk[:, :], WF[:, kt2, mt * 128:(mt + 1) * 128],
                                 RP[:, kt2, :], start=(kt2 == 0),
                                 stop=(kt2 == 1))
            nc.vector.tensor_scalar_add(K2T[:, mt, :], pk[:, :], BF[:, mt, :])
        for jt in range(2):
            pk = ps2_k.tile([128, C], F32, tag="pk")
            for kt2 in range(2):
                nc.tensor.matmul(pk[:, :], RP[:, kt2, jt * 128:(jt + 1) * 128],
                                 WF[:, kt2, :], start=(kt2 == 0),
                                 stop=(kt2 == 1))
            nc.vector.tensor_add(K2K[:, jt, :], pk[:, :], BFr[:, :])

        A2 = a2pool.tile([128, 2, HW], F32R, tag="A2")    # [keys, queries]

        def a2_scores(jt, qb):
            s2 = ps_s2.tile([128, 2048], F32, tag="s2")
            for sub in range(0, 2048, 512):
                for kt2 in range(2):
                    nc.tensor.matmul(
                        s2[:, sub:sub + 512],
                        K2T[:, kt2, jt * 128:(jt + 1) * 128],
                        FW[:, kt2, qb + sub:qb + sub + 512],
                        start=(kt2 == 0), stop=(kt2 == 1))
            nc.scalar.activation(out=A2[:, jt, qb:qb + 2048], in_=s2[:, :],
                                 func=AF.Exp, scale=1.0 / math.sqrt(C))

        def a2_out(qb):
            for qs in range(qb, qb + 2048, QC2):
                d2 = ps2_small.tile([128, QC2], F32, tag="d2")
                for jt in range(2):
                    nc.tensor.matmul(d2[:, :], ONES[:, :],
                                     A2[:, jt, qs:qs + QC2],
                                     start=(jt == 0), stop=(jt == 1))
                dr2 = opool.tile([128, QC2], F32, tag="dr2")
                nc.vector.reciprocal(dr2[:, :], d2[:, :])
                for ct in range(2):
                    f2 = ps2_small.tile([128, QC2], F32, tag="f2")
                    for jt in range(2):
                        nc.tensor.matmul(f2[:, :],
                                         K2K[:, jt, ct * 128:(ct + 1) * 128],
                                         A2[:, jt, qs:qs + QC2],
                                         start=(jt == 0), stop=(jt == 1))
                    tmp = opool.tile([128, QC2], F32, tag="tmp")
                    nc.vector.tensor_mul(tmp[:, :], f2[:, :], dr2[:, :])
                    outc = opool.tile([128, QC2], F32, tag="outc")
                    nc.vector.tensor_add(outc[:, :], tmp[:, :],
                                         FW[:, ct, qs:qs + QC2].bitcast(F32))
                    nc.sync.dma_start(
                        out=o_d[ct * 128:(ct + 1) * 128, qs:qs + QC2],
                        in_=outc[:, :])

        a2_scores(0, 0)
        a2_scores(1, 0)
        a2_scores(0, 2048)
        a2_out(0)
        a2_scores(1, 2048)
        a2_out(2048)


_NC_CACHE = None


def _get_nc():
    global _NC_CACHE
    if _NC_CACHE is None:
        _NC_CACHE = _build()
    return _NC_CACHE


def _prep_inputs(inputs):
    f = np.float32
    s = 1.0 / math.sqrt(HD)
    x = np.asarray(inputs["x"], f).reshape(B, C, HW)
    base = {
        "wq_t": np.ascontiguousarray((np.asarray(inputs["Wq"], f) * s).T),
        "wk_t": np.ascontiguousarray(np.asarray(inputs["Wk"], f).T),
        "wv_t": np.ascontiguousarray(np.asarray(inputs["Wv"], f).T),
        "wo_t": np.ascontiguousarray(np.asarray(inputs["Wo"], f).T),
        "wfuse_t": np.ascontiguousarray(np.asarray(inputs["Wfuse"], f).T),
        "bq": (np.asarray(inputs["bq"], f) * s).reshape(C, 1).copy(),
        "bk": np.asarray(inputs["bk"], f).reshape(C, 1).copy(),
        "bo": np.asarray(inputs["bo"], f).reshape(C, 1).copy(),
        "bfuse": np.asarray(inputs["bfuse"], f).reshape(C, 1).copy(),
        "bv_rep": np.broadcast_to(np.asarray(inputs["bv"], f), (128, C)).copy(),
        "bfuse_rep": np.broadcast_to(np.asarray(inputs["bfuse"], f),
                                     (128, C)).copy(),
        "wrow_rep": np.broadcast_to(np.asarray(inputs["w_row"], f),
                                    (128, h2)).copy(),
        "brow_rep": np.broadcast_to(np.asarray(inputs["b_row"], f),
                                    (128, h2)).copy(),
        "wcol_rep": np.broadcast_to(np.asarray(inputs["w_col"], f),
                                    (128, h2)).copy(),
        "bcol_rep": np.broadcast_to(np.asarray(inputs["b_col"], f),
                                    (128, h2)).copy(),
        "wgt_rep": np.broadcast_to(np.asarray(inputs["w_gt"], f),
                                   (128, H)).copy(),
        "bgt_rep": np.broadcast_to(np.asarray(inputs["b_gt"], f),
                                   (128, H)).copy(),
    }
    return [dict(base, x=np.ascontiguousarray(x[b])) for b in range(B)]


def _run(inputs, **kwargs):
    nc = _get_nc()
    in_maps = _prep_inputs(inputs)
    return run_bass_kernel_spmd(nc, in_maps, core_ids=list(range(B)), **kwargs)


def kernel(**inputs) -> np.ndarray:
    res = _run(inputs)
    out = np.stack([r["o"] for r in res.results], axis=0)
    return out.reshape(B, C, H, W)

